# revision 8
# baseline (speedup 1.0000x reference)
"""Trainium2 Bass kernel for NaiveLSTM (B=64, T=512, D=H=O=1024) on 8 NeuronCores.

Strategy (data-parallel per the sharding hint):
  - Shard batch 8 ways (8 sequences per core), replicate all weights.
  - Host-side marshalling: fuse the 4 gate weight matrices into Wg/Ug [1024, 4096],
    fuse biases, pre-transpose each core's x shard to xT [D, T*8] ((t,b)-minor)
    so the contraction dim lands on SBUF partitions with clean DMAs.
  - Phase A (on device): xg = x @ Wg + bg as a dense M=128-tile GEMM -> DRAM.
  - Phase B: 512 sequential steps; h kept transposed (hT [H,8]) so the recurrent
    GEMM runs with lhsT=hT (weights-side) and streams the SBUF-resident Ug as the
    moving operand.  Gates -> PSUM, ACT sigmoid/tanh, DVE cell update, PE-transpose
    of h back into hT layout.  H is split in two halves so the serial tail of step
    t overlaps the first K-chunks of step t+1.
  - Phase C folded into B: every 32 steps the accumulated hT buffer (SBUF) is used
    as lhsT tiles for y = hs @ W_y + b_y (no DRAM round trip, fills PE gaps).
  - All matmuls use float32r (fp32 storage, reduced-precision multiply at 1 cyc/row).
"""

import numpy as np

_CACHE = {}

B, T, D, H, O = 64, 512, 1024, 1024, 1024
NCORES = 8
BL = B // NCORES          # batch rows per core
NG = 4 * H                # fused gate width
KC = D // 128             # contraction chunks


def _build(t_steps):
    """Build + compile the per-core Bass program for t_steps timesteps."""
    from contextlib import ExitStack
    import concourse.bacc as bacc
    import concourse.tile as tile
    import concourse.mybir as mybir
    import concourse.bass as bass
    from concourse.masks import make_identity

    F32 = mybir.dt.float32
    F32R = mybir.dt.float32r
    AF = mybir.ActivationFunctionType
    assert t_steps % 32 == 0
    TG = t_steps // 32

    nc = bacc.Bacc("TRN2", target_bir_lowering=False, debug=False,
                   num_devices=NCORES)

    xT = nc.dram_tensor("xT", [D, t_steps * BL], F32R, kind="ExternalInput").ap()
    h0T = nc.dram_tensor("h0T", [128, KC * BL], F32R, kind="ExternalInput").ap()
    c0 = nc.dram_tensor("c0", [BL, H], F32, kind="ExternalInput").ap()
    Wg = nc.dram_tensor("Wg", [D, NG], F32R, kind="ExternalInput").ap()
    Ug = nc.dram_tensor("Ug", [H, NG], F32R, kind="ExternalInput").ap()
    bgb = nc.dram_tensor("bgb", [128, NG], F32, kind="ExternalInput").ap()
    Wy = nc.dram_tensor("Wy", [H, O], F32R, kind="ExternalInput").ap()
    byb = nc.dram_tensor("byb", [128, O], F32, kind="ExternalInput").ap()

    y = nc.dram_tensor("y", [t_steps * BL, O], F32, kind="ExternalOutput").ap()
    hT_o = nc.dram_tensor("hT_o", [BL, H], F32, kind="ExternalOutput").ap()
    cT_o = nc.dram_tensor("cT_o", [BL, H], F32, kind="ExternalOutput").ap()

    xg_d = nc.dram_tensor("xg_d", [t_steps * BL, NG], F32, kind="Internal").ap()

    n_mtiles = t_steps * BL // 128

    with tile.TileContext(nc) as tc:
        # ---------------- Phase A: xg = x @ Wg + bg ----------------
        with ExitStack() as ctx:
            wgp = ctx.enter_context(tc.tile_pool(name="wg", bufs=1))
            xap = ctx.enter_context(tc.tile_pool(name="xa", bufs=3))
            bp = ctx.enter_context(tc.tile_pool(name="abias", bufs=1))
            aop = ctx.enter_context(tc.tile_pool(name="aout", bufs=4))
            apsp = ctx.enter_context(tc.tile_pool(name="aps", bufs=4, space="PSUM"))

            wg_sb = wgp.tile([128, KC * NG], F32R)
            for k in range(KC):
                nc.sync.dma_start(wg_sb[:, k * NG:(k + 1) * NG],
                                  Wg[k * 128:(k + 1) * 128, :])
            bg_sb = bp.tile([128, NG], F32)
            nc.sync.dma_start(bg_sb[:], bgb[:, :])

            for m in range(n_mtiles):
                xt = xap.tile([128, KC * 128], F32R)
                for k in range(KC):
                    nc.sync.dma_start(xt[:, k * 128:(k + 1) * 128],
                                      xT[k * 128:(k + 1) * 128,
                                         m * 128:(m + 1) * 128])
                for n in range(8):
                    ps = apsp.tile([128, 512], F32)
                    for k in range(KC):
                        nc.tensor.matmul(
                            ps[:],
                            xt[:, k * 128:(k + 1) * 128],
                            wg_sb[:, k * NG + n * 512: k * NG + (n + 1) * 512],
                            start=(k == 0), stop=(k == KC - 1))
                    ot = aop.tile([128, 512], F32)
                    nc.vector.tensor_add(ot[:], ps[:], bg_sb[:, n * 512:(n + 1) * 512])
                    nc.sync.dma_start(
                        xg_d[m * 128:(m + 1) * 128, n * 512:(n + 1) * 512], ot[:])

        # ---------------- Phase B + C ----------------
        with ExitStack() as ctx:
            ugp = ctx.enter_context(tc.tile_pool(name="ug", bufs=1))
            wyp = ctx.enter_context(tc.tile_pool(name="wy", bufs=3))
            byp = ctx.enter_context(tc.tile_pool(name="bybias", bufs=1))
            idp = ctx.enter_context(tc.tile_pool(name="ident", bufs=1))
            h0p = ctx.enter_context(tc.tile_pool(name="h0", bufs=1))
            cp = ctx.enter_context(tc.tile_pool(name="cstate", bufs=1))
            xgp = ctx.enter_context(tc.tile_pool(name="xgs", bufs=4))
            gp = ctx.enter_context(tc.tile_pool(name="gact", bufs=9))
            tmp = ctx.enter_context(tc.tile_pool(name="tmp", bufs=2))
            thp = ctx.enter_context(tc.tile_pool(name="th", bufs=2))
            hp = ctx.enter_context(tc.tile_pool(name="hh", bufs=3))
            hba_p = ctx.enter_context(tc.tile_pool(name="hbufA", bufs=1))
            hbb_p = ctx.enter_context(tc.tile_pool(name="hbufB", bufs=1))
            yop = ctx.enter_context(tc.tile_pool(name="yout", bufs=2))
            bps = ctx.enter_context(tc.tile_pool(name="bps", bufs=3, space="PSUM"))
            tps = ctx.enter_context(tc.tile_pool(name="tps", bufs=3, space="PSUM"))
            cps = ctx.enter_context(tc.tile_pool(name="cps", bufs=1, space="PSUM"))

            ug_sb = ugp.tile([128, KC * NG], F32R)
            for k in range(KC):
                nc.sync.dma_start(ug_sb[:, k * NG:(k + 1) * NG],
                                  Ug[k * 128:(k + 1) * 128, :])
            by_sb = byp.tile([128, O], F32)
            nc.sync.dma_start(by_sb[:], byb[:, :])

            ident = idp.tile([128, 128], F32)
            make_identity(nc, ident[:])

            h0sb = h0p.tile([128, KC * BL], F32R)
            nc.sync.dma_start(h0sb[:], h0T[:, :])

            c_half = [cp.tile([BL, 512], F32, name=f"c_half{i}") for i in range(2)]
            nc.sync.dma_start(c_half[0][:], c0[:, 0:512])
            nc.sync.dma_start(c_half[1][:], c0[:, 512:1024])

            hbA_prev = hbB_prev = None
            hh_last = [None, None]

            for tg in range(TG):
                # column layout: kk*256 + tt*8 + b  (kk: k-chunk within half)
                hbA = hba_p.tile([128, 32 * 4 * BL], F32R)   # half 0: k=0..3
                hbB = hbb_p.tile([128, 32 * 4 * BL], F32R)   # half 1: k=4..7

                for tt in range(32):
                    t = tg * 32 + tt

                    # lhsT source slices for this step (h_{t-1} transposed)
                    def lhsT(k):
                        if t == 0:
                            return h0sb[:, k * BL:(k + 1) * BL]
                        if tt == 0:
                            src = hbA_prev if k < 4 else hbB_prev
                            return src[:, (k % 4) * 256 + 31 * BL:
                                       (k % 4) * 256 + 32 * BL]
                        src = hbA if k < 4 else hbB
                        return src[:, (k % 4) * 256 + (tt - 1) * BL:
                                   (k % 4) * 256 + tt * BL]

                    # gates: chunk order [j0 j1 i0 i1 f0 f1 o0 o1]
                    gts = []
                    for n in range(8):
                        if n % 2 == 0:
                            xg_t = xgp.tile([BL, NG // 4], F32)
                            nc.sync.dma_start(
                                xg_t[:],
                                xg_d[t * BL:(t + 1) * BL,
                                     (n // 2) * 1024:(n // 2 + 1) * 1024])
                        ps = bps.tile([BL, 512], F32)
                        for k in range(KC):
                            nc.tensor.matmul(
                                ps[:],
                                lhsT(k),
                                ug_sb[:, k * NG + n * 512:
                                      k * NG + (n + 1) * 512],
                                start=(k == 0), stop=(k == KC - 1))
                        gt = gp.tile([BL, 512], F32)
                        nc.vector.tensor_add(gt[:], ps[:],
                                             xg_t[:, (n % 2) * 512:
                                                  (n % 2 + 1) * 512])
                        nc.scalar.activation(gt[:], gt[:],
                                             AF.Tanh if n < 2 else AF.Sigmoid)
                        gts.append(gt)

                    # cell/hidden update per H-half; gts: j=0,1 i=2,3 f=4,5 o=6,7
                    for hf in range(2):
                        c_t = c_half[hf]
                        t1 = tmp.tile([BL, 512], F32)
                        nc.vector.tensor_mul(t1[:], gts[2 + hf][:], gts[0 + hf][:])
                        nc.vector.tensor_mul(c_t[:], gts[4 + hf][:], c_t[:])
                        nc.vector.tensor_add(c_t[:], c_t[:], t1[:])
                        th = thp.tile([BL, 512], F32)
                        nc.scalar.activation(th[:], c_t[:], AF.Tanh)
                        hh = hp.tile([BL, 512], F32)
                        nc.vector.tensor_mul(hh[:], gts[6 + hf][:], th[:])
                        if t == t_steps - 1:
                            hh_last[hf] = hh
                        # transpose h half back into hT layout
                        hb = hbA if hf == 0 else hbB
                        for kk in range(4):
                            tp = tps.tile([128, BL], F32)
                            nc.tensor.transpose(tp[:], hh[:, kk * 128:(kk + 1) * 128],
                                                ident[:BL, :BL])
                            nc.scalar.copy(
                                hb[:, kk * 256 + tt * BL: kk * 256 + (tt + 1) * BL],
                                tp[:])

                # Phase C interleave: y rows for this tgroup
                for n2 in range(2):
                    pss = [cps.tile([128, 512], F32, name=f"cps{i}")
                           for i in range(2)]
                    for k in range(KC):
                        wy_t = wyp.tile([128, 512], F32R)
                        nc.sync.dma_start(
                            wy_t[:], Wy[k * 128:(k + 1) * 128,
                                        n2 * 512:(n2 + 1) * 512])
                        for mh in range(2):
                            hb = hbA if k < 4 else hbB
                            lt = hb[:, (k % 4) * 256 + mh * 128:
                                    (k % 4) * 256 + (mh + 1) * 128]
                            nc.tensor.matmul(
                                pss[mh][:],
                                lt,
                                wy_t[:],
                                start=(k == 0), stop=(k == KC - 1))
                    for mh in range(2):
                        yt = yop.tile([128, 512], F32)
                        nc.vector.tensor_add(yt[:], pss[mh][:],
                                             by_sb[:, n2 * 512:(n2 + 1) * 512])
                        nc.sync.dma_start(
                            y[tg * 256 + mh * 128: tg * 256 + (mh + 1) * 128,
                              n2 * 512:(n2 + 1) * 512], yt[:])

                hbA_prev, hbB_prev = hbA, hbB

            # final state outputs
            for hf in range(2):
                nc.sync.dma_start(hT_o[:, hf * 512:(hf + 1) * 512], hh_last[hf][:])
                nc.sync.dma_start(cT_o[:, hf * 512:(hf + 1) * 512], c_half[hf][:])

    nc.compile()
    return nc


def _marshal(inputs, t_steps):
    """Host-side: fuse weights, shard + transpose per core. Returns in_maps."""
    x = inputs["x"][:, :t_steps, :]
    h0, c0 = inputs["h0"], inputs["c0"]
    Wg = np.concatenate([inputs["W_j"], inputs["W_i"], inputs["W_f"], inputs["W_o"]], axis=1)
    Ug = np.concatenate([inputs["U_j"], inputs["U_i"], inputs["U_f"], inputs["U_o"]], axis=1)
    bg = np.concatenate([inputs["b_ij"] + inputs["b_hj"],
                         inputs["b_ii"] + inputs["b_hi"],
                         inputs["b_if"] + inputs["b_hf"],
                         inputs["b_io"] + inputs["b_ho"]])
    bgb = np.ascontiguousarray(np.broadcast_to(bg, (128, NG)), dtype=np.float32)
    byb = np.ascontiguousarray(np.broadcast_to(inputs["b_y"], (128, O)), dtype=np.float32)
    Wg = np.ascontiguousarray(Wg, dtype=np.float32)
    Ug = np.ascontiguousarray(Ug, dtype=np.float32)
    Wy = np.ascontiguousarray(inputs["W_y"], dtype=np.float32)

    in_maps = []
    for c in range(NCORES):
        sl = slice(c * BL, (c + 1) * BL)
        xk = x[sl]                                     # [BL, T, D]
        xTk = np.ascontiguousarray(
            xk.transpose(2, 1, 0).reshape(D, t_steps * BL), dtype=np.float32)
        h0k = h0[sl]                                   # [BL, H]
        h0Tk = np.ascontiguousarray(
            h0k.T.reshape(KC, 128, BL).transpose(1, 0, 2).reshape(128, KC * BL),
            dtype=np.float32)
        in_maps.append({
            "xT": xTk,
            "h0T": h0Tk,
            "c0": np.ascontiguousarray(c0[sl], dtype=np.float32),
            "Wg": Wg, "Ug": Ug, "bgb": bgb, "Wy": Wy, "byb": byb,
        })
    return in_maps


def _unshard(results, t_steps):
    ys, hs, cs = [], [], []
    for r in results:
        yk = r["y"].reshape(t_steps, BL, O).transpose(1, 0, 2)   # [BL, T, O]
        ys.append(yk)
        hs.append(r["hT_o"])
        cs.append(r["cT_o"])
    y = np.ascontiguousarray(np.concatenate(ys, axis=0))
    h_T = np.concatenate(hs, axis=0)
    c_T = np.concatenate(cs, axis=0)
    return y, h_T, c_T


def kernel(**inputs):
    from concourse.bass_utils import run_bass_kernel_spmd
    t_steps = T
    if t_steps not in _CACHE:
        _CACHE[t_steps] = _build(t_steps)
    nc = _CACHE[t_steps]
    in_maps = _marshal(inputs, t_steps)
    res = run_bass_kernel_spmd(nc, in_maps, core_ids=list(range(NCORES)))
    return _unshard(res.results, t_steps)


# revision 14
# speedup vs baseline: 1.6877x; 1.6877x over previous
"""Trainium2 Bass kernel for NaiveLSTM (B=64, T=512, D=H=O=1024) on 8 NeuronCores.

Strategy (data-parallel per the sharding hint):
  - Shard batch 8 ways (8 sequences per core), replicate all weights.
  - Host-side marshalling: fuse the 4 gate weight matrices into Wg/Ug [1024, 4096],
    fuse biases, pre-transpose each core's x shard to xT [D, T*8] ((t,b)-minor)
    so the contraction dim lands on SBUF partitions with clean DMAs.
  - Phase A (on device): xg = x @ Wg + bg as a dense M=128-tile GEMM -> DRAM.
  - Phase B: 512 sequential steps; h kept transposed (hT [H,8]) so the recurrent
    GEMM runs with lhsT=hT (weights-side) and streams the SBUF-resident Ug as the
    moving operand.  Gates -> PSUM, ACT sigmoid/tanh, DVE cell update, PE-transpose
    of h back into hT layout.  H is split in two halves so the serial tail of step
    t overlaps the first K-chunks of step t+1.
  - Phase C folded into B: every 32 steps the accumulated hT buffer (SBUF) is used
    as lhsT tiles for y = hs @ W_y + b_y (no DRAM round trip, fills PE gaps).
  - All matmuls use float32r (fp32 storage, reduced-precision multiply at 1 cyc/row).
"""

import numpy as np

_CACHE = {}

B, T, D, H, O = 64, 512, 1024, 1024, 1024
NCORES = 8
BL = B // NCORES          # batch rows per core
NG = 4 * H                # fused gate width
KC = D // 128             # contraction chunks


def _build(t_steps):
    """Build + compile the per-core Bass program for t_steps timesteps."""
    from contextlib import ExitStack
    import concourse.bacc as bacc
    import concourse.tile as tile
    import concourse.mybir as mybir
    import concourse.bass as bass
    from concourse.masks import make_identity

    F32 = mybir.dt.float32
    F32R = mybir.dt.float32r
    BF16 = mybir.dt.bfloat16
    AF = mybir.ActivationFunctionType
    assert t_steps % 32 == 0
    TG = t_steps // 32

    nc = bacc.Bacc("TRN2", target_bir_lowering=False, debug=False,
                   num_devices=NCORES)

    xT = nc.dram_tensor("xT", [D, t_steps * BL], F32R, kind="ExternalInput").ap()
    h0T = nc.dram_tensor("h0T", [128, KC * BL], BF16, kind="ExternalInput").ap()
    c0 = nc.dram_tensor("c0", [BL, H], F32, kind="ExternalInput").ap()
    Wg = nc.dram_tensor("Wg", [D, NG], F32R, kind="ExternalInput").ap()
    Ug = nc.dram_tensor("Ug", [H, NG], BF16, kind="ExternalInput").ap()
    bgb = nc.dram_tensor("bgb", [128, NG], F32, kind="ExternalInput").ap()
    Wy = nc.dram_tensor("Wy", [H, O], BF16, kind="ExternalInput").ap()
    byb = nc.dram_tensor("byb", [128, O], F32, kind="ExternalInput").ap()
    qid = nc.dram_tensor("qid", [128, BL], F32, kind="ExternalInput").ap()

    y = nc.dram_tensor("y", [t_steps * BL, O], F32, kind="ExternalOutput").ap()
    hT_o = nc.dram_tensor("hT_o", [BL, H], F32, kind="ExternalOutput").ap()
    cT_o = nc.dram_tensor("cT_o", [BL, H], F32, kind="ExternalOutput").ap()

    xg_d = nc.dram_tensor("xg_d", [t_steps * BL, NG], F32, kind="Internal").ap()

    n_mtiles = t_steps * BL // 128

    with tile.TileContext(nc) as tc:
        # ---------------- Phase A: xg = x @ Wg + bg ----------------
        with ExitStack() as ctx:
            wgp = ctx.enter_context(tc.tile_pool(name="wg", bufs=1))
            xap = ctx.enter_context(tc.tile_pool(name="xa", bufs=3))
            bp = ctx.enter_context(tc.tile_pool(name="abias", bufs=1))
            aop = ctx.enter_context(tc.tile_pool(name="aout", bufs=4))
            apsp = ctx.enter_context(tc.tile_pool(name="aps", bufs=4, space="PSUM"))

            wg_sb = wgp.tile([128, KC * NG], F32R)
            for k in range(KC):
                nc.sync.dma_start(wg_sb[:, k * NG:(k + 1) * NG],
                                  Wg[k * 128:(k + 1) * 128, :])
            bg_sb = bp.tile([128, NG], F32)
            nc.sync.dma_start(bg_sb[:], bgb[:, :])

            for m in range(n_mtiles):
                xt = xap.tile([128, KC * 128], F32R)
                for k in range(KC):
                    nc.sync.dma_start(xt[:, k * 128:(k + 1) * 128],
                                      xT[k * 128:(k + 1) * 128,
                                         m * 128:(m + 1) * 128])
                for n in range(8):
                    ps = apsp.tile([128, 512], F32)
                    for k in range(KC):
                        nc.tensor.matmul(
                            ps[:],
                            xt[:, k * 128:(k + 1) * 128],
                            wg_sb[:, k * NG + n * 512: k * NG + (n + 1) * 512],
                            start=(k == 0), stop=(k == KC - 1))
                    ot = aop.tile([128, 512], F32)
                    nc.vector.tensor_add(ot[:], ps[:], bg_sb[:, n * 512:(n + 1) * 512])
                    nc.sync.dma_start(
                        xg_d[m * 128:(m + 1) * 128, n * 512:(n + 1) * 512], ot[:])

        # ---------------- Phase B + C ----------------
        # Quartered-partition layout: H-quarter q (256 wide) lives on
        # partitions 32q..32q+BL.  The 4 per-K matmuls for one gate run in
        # 4 PE column groups concurrently (tile_position=(0,32q)).
        with ExitStack() as ctx:
            ugp = ctx.enter_context(tc.tile_pool(name="ug", bufs=1))
            wyp = ctx.enter_context(tc.tile_pool(name="wy", bufs=1))
            byp = ctx.enter_context(tc.tile_pool(name="bybias", bufs=1))
            idp = ctx.enter_context(tc.tile_pool(name="ident", bufs=1))
            h0p = ctx.enter_context(tc.tile_pool(name="h0", bufs=1))
            cp = ctx.enter_context(tc.tile_pool(name="cstate", bufs=1))
            xgp = ctx.enter_context(tc.tile_pool(name="xgs", bufs=3))
            gp = ctx.enter_context(tc.tile_pool(name="gact", bufs=6))
            tmp = ctx.enter_context(tc.tile_pool(name="tmp", bufs=2))
            thp = ctx.enter_context(tc.tile_pool(name="th", bufs=2))
            hp = ctx.enter_context(tc.tile_pool(name="hh", bufs=3))
            hba_p = ctx.enter_context(tc.tile_pool(name="hbufA", bufs=1))
            hbb_p = ctx.enter_context(tc.tile_pool(name="hbufB", bufs=1))
            yop = ctx.enter_context(tc.tile_pool(name="yout", bufs=2))
            bps = ctx.enter_context(tc.tile_pool(name="bps", bufs=4, space="PSUM"))
            tps = ctx.enter_context(tc.tile_pool(name="tps", bufs=2, space="PSUM"))
            cps = ctx.enter_context(tc.tile_pool(name="cps", bufs=1, space="PSUM"))

            ug_sb = ugp.tile([128, KC * NG], BF16)
            for k in range(KC):
                nc.sync.dma_start(ug_sb[:, k * NG:(k + 1) * NG],
                                  Ug[k * 128:(k + 1) * 128, :])
            by_sb = byp.tile([128, O], F32)
            nc.sync.dma_start(by_sb[:], byb[:, :])
            wy_sb = wyp.tile([128, KC * O], BF16)
            for k in range(KC):
                nc.sync.dma_start(wy_sb[:, k * O:(k + 1) * O],
                                  Wy[k * 128:(k + 1) * 128, :])

            qid_sb = idp.tile([128, BL], F32)
            nc.sync.dma_start(qid_sb[:], qid[:, :])

            h0sb = h0p.tile([128, KC * BL], BF16)
            nc.sync.dma_start(h0sb[:], h0T[:, :])

            # quartered cell state: quarter q on partitions 32q..32q+BL
            c_q = cp.tile([128, 256], F32)
            c0_v = c0[:, :].rearrange("b (q c) -> b q c", q=4)
            for q in range(4):
                nc.sync.dma_start(c_q[32 * q:32 * q + BL, :], c0_v[:, q, :])

            xg_dv = xg_d[:, :].rearrange("r (g q c) -> r g q c", g=4, q=4)

            hbA_prev = hbB_prev = None
            hh_last = None

            for tg in range(TG):
                # column layout: kk*256 + tt*8 + b  (kk: k-chunk within half)
                hbA = hba_p.tile([128, 32 * 4 * BL], BF16)   # half 0: k=0..3
                hbB = hbb_p.tile([128, 32 * 4 * BL], BF16)   # half 1: k=4..7

                for tt in range(32):
                    t = tg * 32 + tt

                    # lhsT source slices for this step (h_{t-1} transposed)
                    def lhsT(k):
                        if t == 0:
                            return h0sb[:, k * BL:(k + 1) * BL]
                        if tt == 0:
                            src = hbA_prev if k < 4 else hbB_prev
                            return src[:, (k % 4) * 256 + 31 * BL:
                                       (k % 4) * 256 + 32 * BL]
                        src = hbA if k < 4 else hbB
                        return src[:, (k % 4) * 256 + (tt - 1) * BL:
                                   (k % 4) * 256 + tt * BL]

                    # xg for this step, quartered: [128, (g,256)]
                    xg_t = xgp.tile([128, 4 * 256], F32)
                    xg_tv = xg_t[:].rearrange("p (g c) -> p g c", g=4)
                    for q in range(4):
                        nc.sync.dma_start(
                            xg_tv[32 * q:32 * q + BL, :, :],
                            xg_dv[t * BL:(t + 1) * BL, :, q, :])

                    # 4 waves (j, i, f, o); each wave: 4 col-groups x 8 K-chunks
                    gts = []
                    for g in range(4):
                        ps = bps.tile([128, 256], F32)
                        for k in range(KC):
                            lt = lhsT(k)
                            for q in range(4):
                                nc.tensor.matmul(
                                    ps[32 * q:32 * q + BL, :],
                                    lt,
                                    ug_sb[:, k * NG + g * 1024 + q * 256:
                                          k * NG + g * 1024 + (q + 1) * 256],
                                    start=(k == 0), stop=(k == KC - 1),
                                    tile_position=(0, 32 * q))
                        gt = gp.tile([128, 256], F32)
                        nc.vector.tensor_add(gt[:], ps[:],
                                             xg_t[:, g * 256:(g + 1) * 256])
                        nc.scalar.activation(gt[:], gt[:],
                                             AF.Tanh if g == 0 else AF.Sigmoid)
                        gts.append(gt)

                    # cell/hidden update, whole-tile quartered ops
                    t1 = tmp.tile([128, 256], F32)
                    nc.vector.tensor_mul(t1[:], gts[1][:], gts[0][:])      # i*j
                    nc.vector.tensor_mul(c_q[:], gts[2][:], c_q[:])        # f*c
                    nc.vector.tensor_add(c_q[:], c_q[:], t1[:])
                    th = thp.tile([128, 256], F32)
                    nc.scalar.activation(th[:], c_q[:], AF.Tanh)
                    hh = hp.tile([128, 256], F32)
                    nc.vector.tensor_mul(hh[:], gts[3][:], th[:])
                    if t == t_steps - 1:
                        hh_last = hh

                    # transpose h quarters back into hT layout (2 k-chunks/quarter)
                    for q in range(4):
                        for j2 in range(2):
                            k = 2 * q + j2
                            tp = tps.tile([128, BL], F32)
                            nc.tensor.transpose(
                                tp[:],
                                hh[32 * q:32 * q + BL, j2 * 128:(j2 + 1) * 128],
                                qid_sb[32 * q:32 * q + BL, :],
                                tile_position=(32 * q, 0))
                            hb = hbA if k < 4 else hbB
                            nc.scalar.copy(
                                hb[:, (k % 4) * 256 + tt * BL:
                                   (k % 4) * 256 + (tt + 1) * BL],
                                tp[:])

                # Phase C interleave: y rows for this tgroup
                for n2 in range(2):
                    pss = [cps.tile([128, 512], F32, name=f"cps{i}")
                           for i in range(2)]
                    for k in range(KC):
                        for mh in range(2):
                            hb = hbA if k < 4 else hbB
                            lt = hb[:, (k % 4) * 256 + mh * 128:
                                    (k % 4) * 256 + (mh + 1) * 128]
                            nc.tensor.matmul(
                                pss[mh][:],
                                lt,
                                wy_sb[:, k * O + n2 * 512:
                                      k * O + (n2 + 1) * 512],
                                start=(k == 0), stop=(k == KC - 1))
                    for mh in range(2):
                        yt = yop.tile([128, 512], F32)
                        nc.vector.tensor_add(yt[:], pss[mh][:],
                                             by_sb[:, n2 * 512:(n2 + 1) * 512])
                        nc.sync.dma_start(
                            y[tg * 256 + mh * 128: tg * 256 + (mh + 1) * 128,
                              n2 * 512:(n2 + 1) * 512], yt[:])

                hbA_prev, hbB_prev = hbA, hbB

            # final state outputs (quartered -> [BL, H])
            hT_ov = hT_o[:, :].rearrange("b (q c) -> b q c", q=4)
            cT_ov = cT_o[:, :].rearrange("b (q c) -> b q c", q=4)
            for q in range(4):
                nc.sync.dma_start(hT_ov[:, q, :], hh_last[32 * q:32 * q + BL, :])
                nc.sync.dma_start(cT_ov[:, q, :], c_q[32 * q:32 * q + BL, :])

    nc.compile()
    return nc


def _marshal(inputs, t_steps):
    """Host-side: fuse weights, shard + transpose per core. Returns in_maps."""
    x = inputs["x"][:, :t_steps, :]
    h0, c0 = inputs["h0"], inputs["c0"]
    Wg = np.concatenate([inputs["W_j"], inputs["W_i"], inputs["W_f"], inputs["W_o"]], axis=1)
    Ug = np.concatenate([inputs["U_j"], inputs["U_i"], inputs["U_f"], inputs["U_o"]], axis=1)
    bg = np.concatenate([inputs["b_ij"] + inputs["b_hj"],
                         inputs["b_ii"] + inputs["b_hi"],
                         inputs["b_if"] + inputs["b_hf"],
                         inputs["b_io"] + inputs["b_ho"]])
    bgb = np.ascontiguousarray(np.broadcast_to(bg, (128, NG)), dtype=np.float32)
    byb = np.ascontiguousarray(np.broadcast_to(inputs["b_y"], (128, O)), dtype=np.float32)
    import ml_dtypes
    bf16 = ml_dtypes.bfloat16
    Wg = np.ascontiguousarray(Wg, dtype=np.float32)
    Ug = np.ascontiguousarray(Ug).astype(bf16)
    Wy = np.ascontiguousarray(inputs["W_y"]).astype(bf16)

    qid = np.zeros((128, BL), dtype=np.float32)
    for q in range(4):
        for i in range(BL):
            qid[32 * q + i, i] = 1.0

    in_maps = []
    for c in range(NCORES):
        sl = slice(c * BL, (c + 1) * BL)
        xk = x[sl]                                     # [BL, T, D]
        xTk = np.ascontiguousarray(
            xk.transpose(2, 1, 0).reshape(D, t_steps * BL), dtype=np.float32)
        h0k = h0[sl]                                   # [BL, H]
        h0Tk = np.ascontiguousarray(
            h0k.T.reshape(KC, 128, BL).transpose(1, 0, 2).reshape(128, KC * BL),
            dtype=np.float32).astype(bf16)
        in_maps.append({
            "xT": xTk,
            "h0T": h0Tk,
            "c0": np.ascontiguousarray(c0[sl], dtype=np.float32),
            "Wg": Wg, "Ug": Ug, "bgb": bgb, "Wy": Wy, "byb": byb,
            "qid": qid,
        })
    return in_maps


def _unshard(results, t_steps):
    ys, hs, cs = [], [], []
    for r in results:
        yk = r["y"].reshape(t_steps, BL, O).transpose(1, 0, 2)   # [BL, T, O]
        ys.append(yk)
        hs.append(r["hT_o"])
        cs.append(r["cT_o"])
    y = np.ascontiguousarray(np.concatenate(ys, axis=0))
    h_T = np.concatenate(hs, axis=0)
    c_T = np.concatenate(cs, axis=0)
    return y, h_T, c_T


def kernel(**inputs):
    from concourse.bass_utils import run_bass_kernel_spmd
    t_steps = T
    if t_steps not in _CACHE:
        _CACHE[t_steps] = _build(t_steps)
    nc = _CACHE[t_steps]
    in_maps = _marshal(inputs, t_steps)
    res = run_bass_kernel_spmd(nc, in_maps, core_ids=list(range(NCORES)))
    return _unshard(res.results, t_steps)
